# revision 37
# baseline (speedup 1.0000x reference)
"""MGU (minimal gated unit) Bass kernel for Trainium2, 8-core SPMD.

Problem: B=128, T=512, D=U=512 fp32.
    xf = x @ Wf + bf ; xh = x @ Wh + bh            (parallel over B,T)
    scan over t: f = sigmoid(xf_t + h @ Uf)
                 S = tanh(xh_t + (f*h) @ Uh)
                 h = (1-f)*h + f*S
Output: final h [B, U].

Sharding: data-parallel over B (16 rows/core), weights replicated.

Layout ("T-layout"): U (or D) stays on the partition axis, batch on the
free axis, so the sequential recurrence needs no per-step transposes:
  - h/f/S/g tiles: [128p, kt*16b] = [128, 64]   (kt = U/128 = 4)
  - per-step matmul zT[m] = sum_k Uf[k,m].T @ hT[k] -> [128, 4*16] PSUM

Perf structure (~2.66us/step steady state, was 3.23 in the bf16 baseline):
  - Recurrent weights Uf/Uh in fp8e3m4 scaled x128 (relmax ~8e-3 vs the
    2e-2 gate): LDWEIGHTS+MATMUL pairs issue at ~26.7ns, the fp8 FWL
    floor (128 weight cols at 4/cycle on the 1.2GHz xbus). Measured: the
    pace is CLOCK-INDEPENDENT -- step time is identical on both sides of
    the HAM K=8/8 -> 4/8 transition, so PE-warm-keeping fillers are
    pointless. The x128 scale is folded into the bf16 projection weights
    and biases and removed by the free ACT scale=1/128.
  - Dual accumulation (the big one): h' = t2 + t3 with t2 = h-g known
    mid-step and t3 = f*S known only after tanh, so by linearity
        zf(t+1) = seed(xf) + Uf^T t2(t) + Uf^T t3(t)
    and the t2 half of next step's gate-f matmuls issues during the
    zh/tanh window; only the t3 half trails tanh. The h' = t2+t3 add
    leaves the critical chain entirely (h' materializes lazily while
    those matmuls run, in time for the next elementwise ops).
  - Critical chain per step (measured 2660ns cold-steady): sigmoid 313
    -> +33 -> g0 173 -> +55 -> [16 zh MM pairs, 402ns issue span + 174
    retire + 173 PSUM-write latency + ~25 sem] -> tanh 313 -> +33 ->
    t3a 173 -> +55 -> [16 zf t3-part pairs + retire + drain + sem] ->
    next sigmoid. BOTH the g and t3 multiplies are split into k-halves
    feeding k-phased matmul groups (k0,k1 first), so the first 8 matmuls
    start one DVE-half earlier; the ACT ops stay full-width (splitting
    an ACT serializes two ~286ns ops on the FIFO, worse than one 313ns
    op). Every component is at a hardware floor: ACT (N+312)/1.2 fixed
    pipe, DVE ~170ns regardless of 16-64 cols, pair pace = fp8 FWL,
    drain/retire = PE array+PSUM latency, sems ~30-55ns.
  - xf_t/xh_t are seeded into the PSUM accumulator by an identity-weight
    matmul (sets has_written); seeds sit in the PE gaps off the chain.
    The z PSUM banks are explicitly rotated (4 persistent tiles) because
    the pool's most-recently-freed reuse would chain each seed to the
    current sigmoid/tanh bank-read, putting it on the critical edge.
  - f/g/S/t2/t3/h all bf16; t2 = h - g on the idle GpSimd engine.
  - Projection matmuls are emitted via a work queue, half-width (N=256)
    so the scheduler's greedy gap-filling never delays a chain matmul by
    more than ~110ns; the steady-state PSUM->SBUF copies are quartered
    and Vector-only (a Scalar copy in front of a chain sigmoid/tanh
    costs the chain its full duration). The ACT table set is preloaded
    by a dummy sigmoid, and chunk 0's t<4 projections are emitted as
    small slices so the scan starts early; DMAs issue in consumption
    order (gate-f weights + chunk-0 x first, recurrent weights last).
    Measured dead ends (all slower): moving the prologue copies to
    Vector, finer chunk-0 remainder slices + extra pops, reordering the
    prologue DMAs (descriptor-gen is ~600ns serial per queue, so any
    DMA in front of the weights delays them), gpsimd-queue DMAs
    (+250ns/step globally), 6-bank z rotation + seed hoisting (seeds
    already issue the moment their bank's reader ACT completes, off the
    chain edge).

  Remaining time is irreducible on this hardware: ~8us NRT preamble +
  ~8us DMA/first projections, 512 x 2.66us chain steps (ACT pipe,
  fp8-LDW matmul pace, array drain + PSUM-write latency, semaphore
  propagation), ~7us NRT/Tile epilogue (sem teardown).
"""

import os
import numpy as np
import ml_dtypes

import concourse.bass as bass
import concourse.bacc as bacc
import concourse.mybir as mybir
from concourse import tile
from concourse.bass_utils import run_bass_kernel_spmd

B, T, D, U = 128, 512, 512, 512
NCORES = 8
BC = B // NCORES          # batch rows per core = 16
KT = D // 128             # 4 contraction tiles
MT = U // 128             # 4 output tiles
CHUNK = 32                # phase-1 time-chunk; N = CHUNK*BC = 512 per matmul
GW = MT * BC              # scan tile width = 64

SCALE = 128.0             # fp8e3m4 weight pre-scale
INV = 1.0 / SCALE

BF16 = mybir.dt.bfloat16
F8E3 = mybir.dt.float8e3
F32 = mybir.dt.float32
NPBF16 = ml_dtypes.bfloat16
NPF8 = ml_dtypes.float8_e3m4
AF = mybir.ActivationFunctionType
ALU = mybir.AluOpType

_CACHE = {}
LAST_RESULTS = None  # test harness reads exec_time_ns / profile from here


def _build(t_steps: int):
    nc = bacc.Bacc("TRN2", target_bir_lowering=False, debug=False)
    nchunk = (t_steps + CHUNK - 1) // CHUNK

    x_d = nc.dram_tensor("xT", [KT, 128, T * BC], BF16, kind="ExternalInput")
    wf_d = nc.dram_tensor("WfT", [128, KT * U], BF16, kind="ExternalInput")
    wh_d = nc.dram_tensor("WhT", [128, KT * U], BF16, kind="ExternalInput")
    uf_d = nc.dram_tensor("UfT", [128, KT * U], F8E3, kind="ExternalInput")
    uh_d = nc.dram_tensor("UhT", [128, KT * U], F8E3, kind="ExternalInput")
    bf_d = nc.dram_tensor("bfT", [128, MT], F32, kind="ExternalInput")
    bh_d = nc.dram_tensor("bhT", [128, MT], F32, kind="ExternalInput")
    eye_d = nc.dram_tensor("eye", [128, 128], F8E3, kind="ExternalInput")
    out_d = nc.dram_tensor("hT_out", [128, KT * BC], F32, kind="ExternalOutput")

    with tile.TileContext(nc) as tc:
        with (
            tc.tile_pool(name="const", bufs=1) as cpool,
            tc.tile_pool(name="xchunk", bufs=4) as xpool,
            tc.tile_pool(name="proj", bufs=16) as projpool,
            tc.tile_pool(name="work", bufs=4) as wpool,
            tc.tile_pool(name="spsum", bufs=1, space="PSUM") as spsum,
            tc.tile_pool(name="ppsum", bufs=2, space="PSUM") as ppsum,
        ):
            # ---- resident tensors ----
            wf_sb = cpool.tile([128, KT * U], BF16, tag="wf")
            wh_sb = cpool.tile([128, KT * U], BF16, tag="wh")
            uf_sb = cpool.tile([128, KT * U], F8E3, tag="uf")
            uh_sb = cpool.tile([128, KT * U], F8E3, tag="uh")
            bf_sb = cpool.tile([128, MT], F32, tag="bf")
            bh_sb = cpool.tile([128, MT], F32, tag="bh")
            eye_sb = cpool.tile([128, 128], F8E3, tag="eye")

            # preload the sigmoid/tanh ACT table set before the scan needs it
            warm0 = cpool.tile([128, 1], F32, tag="warm0")
            warm1 = cpool.tile([128, 1], F32, tag="warm1")
            nc.vector.memset(warm0[:], 0.0)
            nc.scalar.activation(warm1[:], warm0[:], AF.Sigmoid)

            # per-chunk projection tiles (bf16): free = (t_local, m, b)
            xf_c = [None] * nchunk
            xh_c = [None] * nchunk
            xc_c = [None] * nchunk

            def emit_chunk_dma(c, lo=0, hi=CHUNK, alloc=True):
                if alloc:
                    xc_c[c] = xpool.tile([128, KT * CHUNK * BC], BF16, tag="xc",
                                         name=f"xc{c}")
                    xf_c[c] = projpool.tile([128, CHUNK * GW], BF16, tag="xfc", name=f"xfc{c}")
                    xh_c[c] = projpool.tile([128, CHUNK * GW], BF16, tag="xhc", name=f"xhc{c}")
                xc = xc_c[c]
                CB = CHUNK * BC
                for k in range(KT):
                    nc.sync.dma_start(
                        xc[:, k * CB + lo * BC:k * CB + hi * BC],
                        x_d[k, :, c * CB + lo * BC:c * CB + hi * BC],
                    )

            def proj_group_items(c, gi):
                """One (gate, m) projection group of chunk c as a list of
                closures: 4 matmuls + ACT copy, to be fed into scan PE gaps."""
                gate, m = divmod(gi, MT)
                w_sb, b_sb, dst = ((wf_sb, bf_sb, xf_c[c]), (wh_sb, bh_sb, xh_c[c]))[gate]
                xc = xc_c[c]
                state = {}

                NQ = 2
                HN = CHUNK * BC // NQ

                def mk_mm(k, half):
                    # half-width matmuls: finer PE-queue granularity so a proj
                    # matmul never delays the scan chain by more than ~110ns
                    def emit():
                        if k == 0 and half == 0:
                            state["ps"] = ppsum.tile(
                                [128, CHUNK * BC], F32, tag="pp", name=f"pp{c}_{gi}"
                            )
                        nc.tensor.matmul(
                            state["ps"][:, half * HN:(half + 1) * HN],
                            w_sb[:, k * U + m * 128: k * U + (m + 1) * 128],
                            xc[:, k * CHUNK * BC + half * HN:
                                k * CHUNK * BC + (half + 1) * HN],
                            start=(k == 0), stop=(k == KT - 1),
                            skip_group_check=True,
                        )
                    return emit

                def mk_act():
                    # PSUM->SBUF copy in quarters, Vector only: a Scalar-
                    # engine copy right before a chain sigmoid/tanh costs the
                    # chain its full duration
                    def emit():
                        Q = CHUNK // 4
                        dv = dst[:].rearrange("p (t m b) -> p t m b", t=CHUNK, m=MT, b=BC)
                        pv = state["ps"][:].rearrange("p (t b) -> p t b", t=CHUNK, b=BC)
                        for q in range(4):
                            dq = dv[:, q * Q:(q + 1) * Q, m, :]
                            pq = pv[:, q * Q:(q + 1) * Q, :]
                            nc.vector.tensor_scalar_add(
                                dq, pq, b_sb[:, m:m + 1])
                    return emit

                return [mk_mm(k, hf) for k in range(KT) for hf in range(NQ)] + [mk_act()]

            proj_q = []
            STEP_MS = 0.0015  # conservative per-step sim-time estimate

            def pop_proj(n, t=None):
                if not proj_q:
                    return
                if t is None:
                    for _ in range(n):
                        if proj_q:
                            proj_q.pop(0)()
                    return
                # spread proj work across the scan in simulated time so the
                # scheduler doesn't stuff whole chunks into the nearest gaps
                with tc.tile_wait_until(0.03 + t * STEP_MS):
                    for _ in range(n):
                        if proj_q:
                            proj_q.pop(0)()

            def emit_proj_slice(c, gate, m, ta, tb):
                w_sb, b_sb, dst = ((wf_sb, bf_sb, xf_c[c]), (wh_sb, bh_sb, xh_c[c]))[gate]
                xc = xc_c[c]
                n = (tb - ta) * BC
                ps = ppsum.tile([128, n], F32, tag="pp", name=f"pps{c}_{gate}_{m}_{ta}")
                for k in range(KT):
                    nc.tensor.matmul(
                        ps[:],
                        w_sb[:, k * U + m * 128: k * U + (m + 1) * 128],
                        xc[:, k * CHUNK * BC + ta * BC: k * CHUNK * BC + tb * BC],
                        start=(k == 0), stop=(k == KT - 1),
                        skip_group_check=True,
                    )
                dv = dst[:].rearrange("p (t m b) -> p t m b", t=CHUNK, m=MT, b=BC)
                nc.scalar.activation(
                    dv[:, ta:tb, m, :],
                    ps[:].rearrange("p (t b) -> p t b", t=tb - ta, b=BC),
                    AF.Identity,
                    bias=b_sb[:, m:m + 1],
                )

            MINI = 4
            # prologue: chunk 0's first timesteps in small slices so the scan
            # starts as soon as t=0's projections land; then the remainders
            # DMA order tuned for earliest scan start: gate-f weights and
            # chunk 0 first; recurrent weights are not needed until step 0
            # tiny tensors on the idle Scalar queue: descriptor-gen is
            # ~600ns serial per queue, so on Sync they would push the weight
            # and chunk transfers back
            nc.scalar.dma_start(bf_sb[:], bf_d[:])
            nc.scalar.dma_start(bh_sb[:], bh_d[:])
            nc.scalar.dma_start(eye_sb[:], eye_d[:])
            # Sync queue in strict first-use order; chunk 0's first MINI
            # steps land as a tiny head so the mini projections (and the
            # first sigmoid) don't wait for the full 1MB chunk, and the
            # recurrent weights' transfers complete before step 0 needs them
            nc.sync.dma_start(wf_sb[:], wf_d[:])
            emit_chunk_dma(0, 0, MINI)
            nc.sync.dma_start(wh_sb[:], wh_d[:])
            nc.sync.dma_start(uf_sb[:], uf_d[:])
            nc.sync.dma_start(uh_sb[:], uh_d[:])
            emit_chunk_dma(0, MINI, CHUNK, alloc=False)
            emit_chunk_dma(1)
            for gate in range(2):
                for m in range(MT):
                    emit_proj_slice(0, gate, m, 0, MINI)
            for gate in range(2):
                for m in range(MT):
                    proj_q.append(
                        lambda gate=gate, m=m: emit_proj_slice(0, gate, m, MINI, CHUNK))
            # chunk 1 goes through the queue: emitted inside the loop its
            # items get higher priority numbers than the early steps' chain
            # ops, so the scheduler stops preferring them over the chain
            for gi in range(2 * MT):
                proj_q.extend(proj_group_items(1, gi))
            emit_chunk_dma(2)

            # ---- the sequential scan, with projection work interleaved ----
            h = wpool.tile([128, GW], BF16, tag="h")
            nc.vector.memset(h[:], 0.0)

            def gate_mm(z, u_sb, rhs, k, m, stop=False):
                nc.tensor.matmul(
                    z[:, m * BC:(m + 1) * BC],
                    u_sb[:, k * U + m * 128: k * U + (m + 1) * 128],
                    rhs[:, k * BC:(k + 1) * BC],
                    start=False, stop=stop,
                    skip_group_check=True,
                )

            def gate_accum(z, u_sb, rhs, stop):
                for m in range(MT):
                    for k in range(KT):
                        gate_mm(z, u_sb, rhs, k, m,
                                stop=(stop and m == MT - 1 and k == KT - 1))

            def gate_accum_kphased(z, u_sb, rhs, stop):
                # k-halves 0,1 first: they only need the first half of rhs,
                # which the DVE writes ~160ns before the second half
                for k in (0, 1):
                    for m in range(MT):
                        gate_mm(z, u_sb, rhs, k, m)
                for k in (2, 3):
                    for m in range(MT):
                        gate_mm(z, u_sb, rhs, k, m,
                                stop=(stop and k == KT - 1 and m == MT - 1))

            def seed(z, xsrc):
                nc.tensor.matmul(z[:], eye_sb[:], xsrc, start=True, stop=False,
                                 skip_group_check=True)

            # z tiles are allocated full-bank ([128, 512] fp32) so the 4-deep
            # rotation maps to 4 distinct PSUM banks: the WAR on a bank then
            # trails by 2 steps and the seed matmuls can run inside the
            # ACT/DVE gaps instead of stalling on the current sigmoid/tanh.
            ZB = 512

            # Dual accumulation: h'(t) = t2(t) + t3(t) with t2 = h-g (mid-step,
            # GpSimd) and t3 = f*S (post-tanh). By linearity
            #   zf(t+1) = seed(xf) + Uf^T t2(t) + Uf^T t3(t)
            # so the t2 half of next step's gate-f matmuls issues during the
            # zh/tanh window, and only the t3 half trails the tanh -> the
            # hn add leaves the critical path (h' materializes lazily for the
            # next step's elementwise ops while those matmuls run).

            # explicit 4-bank rotation: the pool's most-recently-freed reuse
            # would make each seed wait on the current sigmoid/tanh bank-read,
            # putting it right on the critical edge of the next matmul phase
            zb = [spsum.tile([128, ZB], F32, tag=f"zb{i}", name=f"zb{i}")
                  for i in range(4)]
            zf = zb[0]
            seed(zf[:, 0:GW], xf_c[0][:, 0:GW])
            t3_prev = None

            for t in range(t_steps):
                c, tl = divmod(t, CHUNK)
                nxt = c + 2
                if tl == 0:
                    # DMA one chunk further ahead than the proj enqueue so the
                    # transfer never completes (and releases a 40-item burst of
                    # ready proj work) right at a chunk-boundary step
                    if c + 3 < nchunk:
                        emit_chunk_dma(c + 3)
                    if nxt < nchunk:
                        for gi in range(2 * MT):
                            proj_q.extend(proj_group_items(nxt, gi))

                # trailing (on-chain) half of this step's gate-f matmuls
                if t3_prev is not None:
                    gate_accum_kphased(zf[:, 0:GW], uf_sb, t3_prev, stop=True)

                zh = zb[(2 * t + 1) % 4]
                seed(zh[:, 0:GW], xh_c[c][:, tl * GW:(tl + 1) * GW])

                f = wpool.tile([128, GW], BF16, tag="f")
                nc.scalar.activation(f[:], zf[:, 0:GW], AF.Sigmoid, scale=INV)
                # g in k-halves: the first half unblocks zh's k01 matmuls
                # one DVE-half earlier
                HW = GW // 2
                g = wpool.tile([128, GW], BF16, tag="g")
                nc.vector.tensor_tensor(g[:, 0:HW], f[:, 0:HW], h[:, 0:HW], ALU.mult)
                nc.vector.tensor_tensor(g[:, HW:GW], f[:, HW:GW], h[:, HW:GW], ALU.mult)
                t2 = wpool.tile([128, GW], BF16, tag="t2")
                nc.gpsimd.tensor_tensor(t2[:], h[:], g[:], ALU.subtract)

                gate_accum_kphased(zh[:, 0:GW], uh_sb, g, stop=True)
                pop_proj(1)

                # next step's zf: seed + the t2 half of its gate-f matmuls,
                # all inside the tanh/update window
                if t + 1 < t_steps:
                    c1, tl1 = divmod(t + 1, CHUNK)
                    zf = zb[(2 * t + 2) % 4]
                    seed(zf[:, 0:GW], xf_c[c1][:, tl1 * GW:(tl1 + 1) * GW])
                    gate_accum(zf[:, 0:GW], uf_sb, t2, stop=False)
                pop_proj(2)

                s = wpool.tile([128, GW], BF16, tag="s")
                nc.scalar.activation(s[:], zh[:, 0:GW], AF.Tanh, scale=INV)
                # t3 in k-halves (like g): the first half unblocks next
                # step's zf k01 matmuls one DVE-half earlier
                t3 = wpool.tile([128, GW], BF16, tag="t3")
                nc.vector.tensor_tensor(t3[:, 0:HW], f[:, 0:HW], s[:, 0:HW], ALU.mult)
                nc.vector.tensor_tensor(t3[:, HW:GW], f[:, HW:GW], s[:, HW:GW], ALU.mult)

                # h' = t2 + f*S   (off the critical path)
                last = (t == t_steps - 1)
                hn = wpool.tile([128, GW], F32 if last else BF16, tag="hout" if last else "h")
                nc.vector.tensor_tensor(hn[:], t2[:], t3[:], ALU.add)
                h = hn
                t3_prev = t3

            pop_proj(len(proj_q))
            nc.sync.dma_start(out_d[:], h[:])

    nc.compile()
    return nc


def _prep_weight_t(w, dtype):
    # [D, U] fp32 -> [128, KT*U] with [:, k*U+m] = w[k*128+p, m]
    return np.ascontiguousarray(
        w.reshape(KT, 128, U).transpose(1, 0, 2).reshape(128, KT * U)
    ).astype(dtype)


def kernel(x, Wf, Uf, bf, Wh, Uh, bh):
    global LAST_RESULTS
    x = np.asarray(x, dtype=np.float32)
    Wf = np.asarray(Wf, dtype=np.float32)
    Uf = np.asarray(Uf, dtype=np.float32)
    Wh = np.asarray(Wh, dtype=np.float32)
    Uh = np.asarray(Uh, dtype=np.float32)
    bf = np.asarray(bf, dtype=np.float32)
    bh = np.asarray(bh, dtype=np.float32)

    t_steps = int(os.environ.get("BASS_MGU_T", T))
    if t_steps not in _CACHE:
        _CACHE[t_steps] = _build(t_steps)
    nc = _CACHE[t_steps]

    wf_t = _prep_weight_t(Wf * SCALE, NPBF16)
    wh_t = _prep_weight_t(Wh * SCALE, NPBF16)
    uf_t = _prep_weight_t(np.clip(Uf * SCALE, -15.5, 15.5), NPF8)
    uh_t = _prep_weight_t(np.clip(Uh * SCALE, -15.5, 15.5), NPF8)
    bf_t = np.ascontiguousarray((bf * SCALE).reshape(MT, 128).T).astype(np.float32)
    bh_t = np.ascontiguousarray((bh * SCALE).reshape(MT, 128).T).astype(np.float32)
    eye = np.eye(128, dtype=np.float32).astype(NPF8)

    in_maps = []
    for ci in range(NCORES):
        xc = x[ci * BC:(ci + 1) * BC]                       # [BC, T, D]
        xt = xc.transpose(2, 1, 0)                          # [D, T, BC]
        xt = np.ascontiguousarray(xt.reshape(KT, 128, T * BC)).astype(NPBF16)
        in_maps.append({
            "xT": xt, "WfT": wf_t, "WhT": wh_t, "UfT": uf_t, "UhT": uh_t,
            "bfT": bf_t, "bhT": bh_t, "eye": eye,
        })

    trace = bool(int(os.environ.get("BASS_MGU_TRACE", "0")))
    kw = {}
    if trace and os.environ.get("BASS_TRACE_DIR"):
        kw["tmpdir"] = os.environ["BASS_TRACE_DIR"]
    res = run_bass_kernel_spmd(nc, in_maps, list(range(NCORES)), trace=trace, **kw)
    LAST_RESULTS = res

    out = np.empty((B, U), dtype=np.float32)
    for ci in range(NCORES):
        ho = np.asarray(res.results[ci]["hT_out"])          # [128, KT*BC]
        out[ci * BC:(ci + 1) * BC] = (
            ho.reshape(128, KT, BC).transpose(2, 1, 0).reshape(BC, U)
        )
    return out

